# revision 63
# baseline (speedup 1.0000x reference)
"""Trainium2 Bass kernel for nn_MediumRangeEdge (retrieval_knn).

For each batch graph: L2-normalize node features, pairwise score
score = sim - (rel + INF*mask)/2 (row/col constants dropped: ||x||=1),
top-10 largest per node, emit edge list [dst, src, 0].

Distribution: data-parallel over batch. 32 graphs -> 8 NeuronCores, 4
graphs per core. No cross-device communication.

Packed-score top-k (the key trick): the device packs the column index
into the low bits of the score so NO max_index / full-width
match_replace passes are needed:

    packed[n, m] = q1024(S*sim[n,m]) - q1024(S*cb[n,m]) + (1023 - m)

with S = 2^23 and q1024 = round-to-multiple-of-1024. All three terms are
exact integers < 2^24 in f32, so the low 10 bits carry the index and the
host decodes m = 1023 - (packed mod 1024). Ordering by packed ==
ordering by quantized score with ties broken low-index-first, matching
jax.lax.top_k. Quantization at delta = 1024/S = 1.2e-4 plus bf16 matmul
inputs scrambles only near-ties (measured rel_err ~1.8e-3, gate 2e-2).

How each term is produced:
  - PE: sim via bf16 matmuls (inputs pre-scaled by sqrt(S) on host, so
    PSUM = S*sim), then ONE extra rank-1 matmul row accumulates
    C = 1.5*2^33 LAST: the f32 add in the [2^33, 2^34) binade rounds
    S*sim to a multiple of 1024 (the quantizer is the PSUM itself).
  - ACT: Identity-activation copy PSUM->SBUF with bias = -C (per-
    partition AP) removes C exactly, leaving praw = q1024(S*sim).
  - Pool+DVE: the per-tile subtract of the host-built constant
    cb3[n, m] = q1024(S*cb[n,m]) - (1023 - m) is column-split at SPLIT:
    Pool takes cols [0:SPLIT] (Q7 software sub, 0.42 roofline), DVE the
    rest, sized so Pool's share equals DVE's share plus the top-k that
    only DVE can run -- all of PE/Pool/DVE sit within ~1.5us of each
    other (~35us busy each).
  - DVE top-k per 128-row tile: max8 over even columns, max8 over odd
    columns (spatial clusters split ~5/5 between parities so per-parity
    top-8 covers the global top-10 with ~0.1% tail misses), then
    max8 / match_replace / max8 on the 16 merged candidates. 5 DVE ops
    of which only two scan 392 elements -- ~1.2us/tile vs 4.4us for the
    classic 5-pass full-width top-k.

P = sim matrix is symmetric: only 256-wide column blocks not fully below
the diagonal are computed; fully-below blocks and the 16-row tail are
mirrored from earlier row tiles with PE transposes (praw is quantized
identically on both sides, so mirrors are bit-consistent).

Host does layout only: normalize + transpose + bf16-cast of the inputs,
the cb3 constant, and final index decode + edge-list assembly.
"""

import sys

if "/opt/trn_rl_repo" not in sys.path:
    sys.path.insert(0, "/opt/trn_rl_repo")

import numpy as np

BATCH = 32
N = 784  # 28*28 nodes
D = 512
K = 10
RES = 28
INF = 100000.0
NCORES = 8
BPC = BATCH // NCORES  # graphs per core

P = 128
NRT = 6  # full 128-row tiles; tail 16 rows handled packed across graphs
S = float(2.0**23)
C = float(1.5 * 2.0**33)  # binade [2^33, 2^34): f32 add rounds to 1024-multiples

# Every tile's packed-subtract is split at column SPLIT between Pool
# (cols [0:SPLIT], Q7 software sub at 0.42 roofline) and DVE (the rest).
# SPLIT is chosen so Pool's share matches DVE's share PLUS the 1.17us
# top-k that only DVE can run -- both engines stay saturated every tile.
SPLIT = 648
# tail-tile sub split (Pool part runs while DVE drains its topk backlog)
SPLIT6 = 648
# per-graph tile order: rt5 before rt4 (rt5's mirrors only need rt0-3) so the
# last full tile's sub+topk overlaps rt4's matmuls instead of trailing them
RT_ORDER = [0, 1, 2, 3, 5, 4]
# dummy 512-wide rank-1 matmuls at kernel start to warm the PE p-state model
N_WARMUP = 0

_CACHE = {}


def _mask_np():
    idx = np.arange(N)
    r, c = idx // RES, idx % RES
    mask = np.zeros((N, N), np.float32)
    for dr, dc in [(0, -1), (0, 1), (-1, 0), (1, 0), (-1, -1), (-1, 1), (1, -1), (1, 1)]:
        rr, cc = r + dr, c + dc
        valid = (rr >= 0) & (rr < RES) & (cc >= 0) & (cc < RES)
        mask[idx[valid], (rr * RES + cc)[valid]] = 1.0
    mask[idx, idx] = 1.0
    return mask


def build_bass():
    import concourse.bacc as bacc
    import concourse.mybir as mybir
    from concourse.tile import TileContext
    from concourse.masks import make_identity
    from contextlib import ExitStack

    f32 = mybir.dt.float32
    bf16 = mybir.dt.bfloat16
    AF = mybir.ActivationFunctionType
    AL = mybir.AluOpType

    nc = bacc.Bacc("TRN2", target_bir_lowering=False, debug=False, num_devices=NCORES)
    xhT = nc.declare_dram_parameter("xhT", [BPC, D, N], bf16, isOutput=False)
    cb3 = nc.declare_dram_parameter("cb3", [N, N], f32, isOutput=False)
    res_out = nc.declare_dram_parameter(
        "res", [P, BPC * NRT * 16 + 16], f32, isOutput=True
    )

    with TileContext(nc) as tc, ExitStack() as ctx:
        consts = ctx.enter_context(tc.tile_pool(name="consts", bufs=1))
        xh_pool = ctx.enter_context(tc.tile_pool(name="xh", bufs=16))
        praw_pool = ctx.enter_context(tc.tile_pool(name="praw", bufs=14))
        packed_pool = ctx.enter_context(tc.tile_pool(name="packed", bufs=6))
        v_pool = ctx.enter_context(tc.tile_pool(name="v16", bufs=10))
        ps_mm = ctx.enter_context(tc.tile_pool(name="ps_mm", bufs=4, space="PSUM"))

        ident = consts.tile([P, P], f32, name="ident")
        make_identity(nc, ident)
        ones = consts.tile([1, P], bf16, name="ones")
        crow = consts.tile([1, 512], bf16, name="crow")
        cneg = consts.tile([P, 1], f32, name="cneg")
        nc.gpsimd.memset(ones, 1.0)
        nc.gpsimd.memset(crow, C)
        nc.gpsimd.memset(cneg, -C)

        # bias constant: rows rt*128+p at block rt (full tiles only).
        # One tile per rt (not one big tile): the Tile framework tracks
        # dependencies at tile granularity, so a single shared tile would make
        # the FIRST subtract wait for ALL bias DMAs. HWDGE/DMA_ENGINES are
        # serialized global devices, so issue order below matters too.
        cb_t = [consts.tile([P, N], f32, name=f"cb_{rt}") for rt in range(NRT)]
        cb6 = consts.tile([P, N], f32, name="cb6")

        def load_cb(rt):
            nc.sync.dma_start(
                out=cb_t[rt], in_=cb3.ap()[rt * P : (rt + 1) * P, :]
            )

        def load_cb6(b):
            # tail-tile bias: graph b occupies partitions 32b..32b+16
            nc.sync.dma_start(
                out=cb6[32 * b : 32 * b + 16, :], in_=cb3.ap()[NRT * P :, :]
            )

        staging = consts.tile([P, BPC * NRT * 16 + 16], f32, name="staging")
        praw6 = consts.tile([P, N], f32, name="praw6")
        praw_t = [dict() for _ in range(BPC)]
        xh_t = {}

        def topk(packed, out16):
            """per-partition top-10 of packed[*, 0:784] -> out16 (sorted 16)."""
            v16 = v_pool.tile([P, 16], f32, tag="v16")
            pv = packed.rearrange("p (m two) -> p two m", two=2)
            nc.vector.max(out=v16[:, 0:8], in_=pv[:, 0])
            nc.vector.max(out=v16[:, 8:16], in_=pv[:, 1])
            nc.vector.max(out=out16[:, 0:8], in_=v16)
            nc.vector.match_replace(
                out=v16, in_to_replace=out16[:, 0:8], in_values=v16,
                imm_value=-3.0e38,
            )
            nc.vector.max(out=out16[:, 8:16], in_=v16)

        def pe_warmup():
            # The cost model halves PE speed for ~3us after an idle gap.
            # Burn dummy rank-1 matmuls into a scratch psum bank while the
            # first xh DMA is in flight so the real matmuls start warm.
            wps = ps_mm.tile([P, 1024], f32, tag="ps_mm", name="warmup_ps")
            for i in range(N_WARMUP):
                nc.tensor.matmul(
                    wps[:, 0:512], lhsT=ones, rhs=crow,
                    start=True, stop=True,
                )

        def load_graph(b):
            # one tile per contraction k-block so the first matmuls only wait
            # on the first 1/4 of the transfer
            xk = []
            for k in range(4):
                t = xh_pool.tile([P, N], bf16, tag="xh", name=f"xh_{b}_{k}")
                nc.sync.dma_start(
                    out=t, in_=xhT.ap()[b, k * P : (k + 1) * P, :]
                )
                xk.append(t)
            xh_t[b] = xk

        def rt_tile(b, rt):
            xh = xh_t[b]
            t = b * NRT + rt
            n_mirror = rt // 2
            d0 = n_mirror * 256
            psum = ps_mm.tile([P, 1024], f32, tag="ps_mm")
            for c in range(n_mirror, 3):
                sl = psum[:, c * 256 : (c + 1) * 256]
                for k in range(4):
                    nc.tensor.matmul(
                        sl,
                        lhsT=xh[k][:, rt * P : rt * P + P],
                        rhs=xh[k][:, c * 256 : (c + 1) * 256],
                        start=(k == 0),
                        stop=False,
                    )
                nc.tensor.matmul(
                    sl, lhsT=ones, rhs=crow[:, 0:256], start=False, stop=True
                )
            sl = psum[:, 768:784]
            for k in range(4):
                nc.tensor.matmul(
                    sl,
                    lhsT=xh[k][:, rt * P : rt * P + P],
                    rhs=xh[k][:, 768:784],
                    start=(k == 0),
                    stop=False,
                )
            nc.tensor.matmul(sl, lhsT=ones, rhs=crow[:, 0:16], start=False, stop=True)

            praw = praw_pool.tile([P, N], f32, tag="praw", name=f"praw_{b}_{rt}")
            praw_t[b][rt] = praw
            # mirrored below-diagonal blocks transpose into the unused low
            # region of the same psum tile (no separate transpose-psum pool)
            for mi in range(2 * n_mirror):
                nc.tensor.transpose(
                    psum[:, mi * P : (mi + 1) * P],
                    praw_t[b][mi][:, rt * P : (rt + 1) * P],
                    ident,
                )
            if b == BPC - 1 and rt >= 4:
                # last graph's final tiles: emit the 16-wide tail slice first
                # so the rt6 mirror transpose (tail critical path) starts as
                # soon as the psum is ready
                nc.scalar.activation(
                    praw[:, 768:784], psum[:, 768:784], AF.Identity, bias=cneg
                )
                nc.scalar.activation(
                    praw[:, d0:768], psum[:, d0:768], AF.Identity, bias=cneg
                )
            else:
                nc.scalar.activation(
                    praw[:, d0:784], psum[:, d0:784], AF.Identity, bias=cneg
                )
            if n_mirror:
                nc.scalar.activation(praw[:, 0:d0], psum[:, 0:d0], AF.Copy)
            packed = packed_pool.tile([P, N], f32, tag="packed")
            nc.gpsimd.tensor_sub(
                packed[:, 0:SPLIT], praw[:, 0:SPLIT], cb_t[rt][:, 0:SPLIT]
            )
            nc.vector.tensor_sub(
                packed[:, SPLIT:784], praw[:, SPLIT:784], cb_t[rt][:, SPLIT:784]
            )
            topk(packed, staging[:, t * 16 : (t + 1) * 16])

        def rt6_graph(b):
            """tail rows 768:784 of graph b -> praw6[32b:32b+16]."""
            xh = xh_t[b]
            r0 = 32 * b
            ps6 = ps_mm.tile([P, 1024], f32, tag="ps_mm", name=f"ps6_{b}")
            for mt in range(NRT):
                nc.tensor.transpose(
                    ps6[:16, mt * P : (mt + 1) * P],
                    praw_t[b][mt][:, 768:784],
                    ident,
                )
            sl = ps6[:16, 768:784]
            for k in range(4):
                nc.tensor.matmul(
                    sl,
                    lhsT=xh[k][:, 768:784],
                    rhs=xh[k][:, 768:784],
                    start=(k == 0),
                    stop=False,
                )
            nc.tensor.matmul(
                sl, lhsT=ones[:, 0:16], rhs=crow[:, 0:16], start=False, stop=True
            )
            if b == BPC - 1:
                # split by mirror-source readiness: mt0-3 land early, mt5
                # after tile (b,5), mt4 (cols 512:640) is the tail critical
                # path -- keep its ACT slice minimal
                nc.scalar.activation(
                    praw6[r0 : r0 + 16, 0:512], ps6[:16, 0:512], AF.Copy
                )
                nc.scalar.activation(
                    praw6[r0 : r0 + 16, 640:768], ps6[:16, 640:768], AF.Copy
                )
                nc.scalar.activation(
                    praw6[r0 : r0 + 16, 512:640], ps6[:16, 512:640], AF.Copy
                )
            else:
                nc.scalar.activation(
                    praw6[r0 : r0 + 16, 0:768], ps6[:16, 0:768], AF.Copy
                )
            nc.scalar.activation(
                praw6[r0 : r0 + 16, 768:784], ps6[:16, 768:784],
                AF.Identity, bias=cneg[:16],
            )

        # DMA issue order matters: xh of graph 0 first so compute starts
        # immediately, bias blocks interleaved in consumption order.
        if N_WARMUP:
            pe_warmup()
        load_graph(0)
        load_cb(0)
        load_cb(1)
        load_graph(1)
        load_cb(2)
        load_cb(3)
        load_graph(2)
        load_cb(4)
        load_cb(5)
        load_graph(3)
        for b in range(BPC):
            load_cb6(b)
        for b in range(BPC):
            for rt in RT_ORDER:
                rt_tile(b, rt)
            rt6_graph(b)
            if b == 1:
                # first half of the results ships mid-run; the final DMA
                # then only moves 208 columns, shrinking the end epilogue
                nc.sync.dma_start(
                    out=res_out.ap()[:, 0:192], in_=staging[:, 0:192]
                )

        packed6 = packed_pool.tile([P, N], f32, tag="packed", name="packed6")
        nc.gpsimd.tensor_sub(
            packed6[:, 0:SPLIT6], praw6[:, 0:SPLIT6], cb6[:, 0:SPLIT6]
        )
        nc.vector.tensor_sub(
            packed6[:, SPLIT6:784], praw6[:, SPLIT6:784], cb6[:, SPLIT6:784]
        )
        topk(packed6, staging[:, BPC * NRT * 16 :])

        nc.sync.dma_start(out=res_out.ap()[:, 192:], in_=staging[:, 192:])

    nc.finalize()
    return nc


def _get_nc():
    if "nc" not in _CACHE:
        _CACHE["nc"] = build_bass()
    return _CACHE["nc"]


def kernel(node_feature, relative_pos):
    import ml_dtypes
    from concourse.bass_utils import run_bass_kernel_spmd

    x = np.asarray(node_feature, dtype=np.float32)
    rel = np.asarray(relative_pos, dtype=np.float32).reshape(N, N)

    # host prep: normalize, scale by sqrt(S), transpose to [D, N], bf16
    nrm = np.sqrt((x * x).sum(-1, dtype=np.float32), dtype=np.float32)
    nrm = np.maximum(nrm, np.float32(1e-12))
    xh = (x / nrm[..., None]) * np.float32(np.sqrt(S))
    xhT = np.ascontiguousarray(xh.transpose(0, 2, 1)).astype(ml_dtypes.bfloat16)

    # cb3 = q1024(S*cb) - (1023 - m),   cb = (rel + INF*mask)/2
    cb = (rel + np.float32(INF) * _mask_np()) * np.float32(0.5)
    q_cb = np.rint(cb.astype(np.float64) * S / 1024.0) * 1024.0
    r_m = (1023 - np.arange(N, dtype=np.float64))[None, :]
    cb3 = (q_cb - r_m).astype(np.float32)

    nc = _get_nc()
    in_maps = [
        {
            "xhT": np.ascontiguousarray(xhT[i * BPC : (i + 1) * BPC]),
            "cb3": cb3,
        }
        for i in range(NCORES)
    ]
    res = run_bass_kernel_spmd(nc, in_maps, list(range(NCORES)))

    # decode: packed -> column index, take top-10 of the sorted 16
    topk = np.zeros((BATCH, N, K), np.int32)
    for i in range(NCORES):
        r = res.results[i]["res"].astype(np.float64)  # [128, BPC*6*16 + 16]
        for b in range(BPC):
            g = i * BPC + b
            for rt in range(NRT):
                t = b * NRT + rt
                pk = r[:, t * 16 : t * 16 + K]
                topk[g, rt * P : (rt + 1) * P, :] = (
                    1023.0 - np.mod(pk, 1024.0)
                ).astype(np.int32)
            pk6 = r[32 * b : 32 * b + 16, BPC * NRT * 16 : BPC * NRT * 16 + K]
            topk[g, NRT * P :, :] = (1023.0 - np.mod(pk6, 1024.0)).astype(np.int32)

    dst = topk + (np.arange(BATCH, dtype=np.int32) * N)[:, None, None]
    src = np.broadcast_to(
        np.arange(BATCH * N, dtype=np.int32).reshape(BATCH, N, 1), (BATCH, N, K)
    )
    relation = np.zeros_like(dst)
    return np.stack([dst, src, relation], axis=-1).reshape(-1, 3)


# revision 65
# speedup vs baseline: 1.0128x; 1.0128x over previous
"""Trainium2 Bass kernel for nn_MediumRangeEdge (retrieval_knn).

For each batch graph: L2-normalize node features, pairwise score
score = sim - (rel + INF*mask)/2 (row/col constants dropped: ||x||=1),
top-10 largest per node, emit edge list [dst, src, 0].

Distribution: data-parallel over batch. 32 graphs -> 8 NeuronCores, 4
graphs per core. No cross-device communication.

Packed-score top-k (the key trick): the device packs the column index
into the low bits of the score so NO max_index / full-width
match_replace passes are needed:

    packed[n, m] = q1024(S*sim[n,m]) - q1024(S*cb[n,m]) + (1023 - m)

with S = 2^23 and q1024 = round-to-multiple-of-1024. All three terms are
exact integers < 2^24 in f32, so the low 10 bits carry the index and the
host decodes m = 1023 - (packed mod 1024). Ordering by packed ==
ordering by quantized score with ties broken low-index-first, matching
jax.lax.top_k. Quantization at delta = 1024/S = 1.2e-4 plus bf16 matmul
inputs scrambles only near-ties (measured rel_err ~1.8e-3, gate 2e-2).

How each term is produced:
  - PE: sim via bf16 matmuls (inputs pre-scaled by sqrt(S) on host, so
    PSUM = S*sim), then ONE extra rank-1 matmul row accumulates
    C = 1.5*2^33 LAST: the f32 add in the [2^33, 2^34) binade rounds
    S*sim to a multiple of 1024 (the quantizer is the PSUM itself).
  - ACT: Identity-activation copy PSUM->SBUF with bias = -C (per-
    partition AP) removes C exactly, leaving praw = q1024(S*sim).
  - Pool+DVE: the per-tile subtract of the host-built constant
    cb3[n, m] = q1024(S*cb[n,m]) - (1023 - m) is column-split at SPLIT:
    Pool takes cols [0:SPLIT] (Q7 software sub, 0.42 roofline), DVE the
    rest, sized so Pool's share equals DVE's share plus the top-k that
    only DVE can run -- all of PE/Pool/DVE sit within ~1.5us of each
    other (~35us busy each).
  - DVE top-k per 128-row tile: max8 over even columns, max8 over odd
    columns (spatial clusters split ~5/5 between parities so per-parity
    top-8 covers the global top-10 with ~0.1% tail misses), then
    max8 / match_replace / max8 on the 16 merged candidates. 5 DVE ops
    of which only two scan 392 elements -- ~1.2us/tile vs 4.4us for the
    classic 5-pass full-width top-k.

P = sim matrix is symmetric: only 256-wide column blocks not fully below
the diagonal are computed; fully-below blocks and the 16-row tail are
mirrored from earlier row tiles with PE transposes (praw is quantized
identically on both sides, so mirrors are bit-consistent).

Host does layout only: normalize + transpose + bf16-cast of the inputs,
the cb3 constant, and final index decode + edge-list assembly.
"""

import sys

if "/opt/trn_rl_repo" not in sys.path:
    sys.path.insert(0, "/opt/trn_rl_repo")

import numpy as np

BATCH = 32
N = 784  # 28*28 nodes
D = 512
K = 10
RES = 28
INF = 100000.0
NCORES = 8
BPC = BATCH // NCORES  # graphs per core

P = 128
NRT = 6  # full 128-row tiles; tail 16 rows handled packed across graphs
S = float(2.0**23)
C = float(1.5 * 2.0**33)  # binade [2^33, 2^34): f32 add rounds to 1024-multiples

# Every tile's packed-subtract is split at column SPLIT between Pool
# (cols [0:SPLIT], Q7 software sub at 0.42 roofline) and DVE (the rest).
# SPLIT is chosen so Pool's share matches DVE's share PLUS the 1.17us
# top-k that only DVE can run -- both engines stay saturated every tile.
SPLIT = 616
# tail-tile sub split (Pool part runs while DVE drains its topk backlog)
SPLIT6 = 648
# natural per-graph tile order (an rt5-before-rt4 variant helped an older
# structure but costs ~0.6us under the split-sub pipeline)
RT_ORDER = [0, 1, 2, 3, 4, 5]
# dummy 512-wide rank-1 matmuls at kernel start to warm the PE p-state model
N_WARMUP = 0

_CACHE = {}


def _mask_np():
    idx = np.arange(N)
    r, c = idx // RES, idx % RES
    mask = np.zeros((N, N), np.float32)
    for dr, dc in [(0, -1), (0, 1), (-1, 0), (1, 0), (-1, -1), (-1, 1), (1, -1), (1, 1)]:
        rr, cc = r + dr, c + dc
        valid = (rr >= 0) & (rr < RES) & (cc >= 0) & (cc < RES)
        mask[idx[valid], (rr * RES + cc)[valid]] = 1.0
    mask[idx, idx] = 1.0
    return mask


def build_bass():
    import concourse.bacc as bacc
    import concourse.mybir as mybir
    from concourse.tile import TileContext
    from concourse.masks import make_identity
    from contextlib import ExitStack

    f32 = mybir.dt.float32
    bf16 = mybir.dt.bfloat16
    AF = mybir.ActivationFunctionType
    AL = mybir.AluOpType

    nc = bacc.Bacc("TRN2", target_bir_lowering=False, debug=False, num_devices=NCORES)
    xhT = nc.declare_dram_parameter("xhT", [BPC, D, N], bf16, isOutput=False)
    cb3 = nc.declare_dram_parameter("cb3", [N, N], f32, isOutput=False)
    res_out = nc.declare_dram_parameter(
        "res", [P, BPC * NRT * 16 + 16], f32, isOutput=True
    )

    with TileContext(nc) as tc, ExitStack() as ctx:
        consts = ctx.enter_context(tc.tile_pool(name="consts", bufs=1))
        xh_pool = ctx.enter_context(tc.tile_pool(name="xh", bufs=16))
        praw_pool = ctx.enter_context(tc.tile_pool(name="praw", bufs=14))
        packed_pool = ctx.enter_context(tc.tile_pool(name="packed", bufs=6))
        v_pool = ctx.enter_context(tc.tile_pool(name="v16", bufs=10))
        ps_mm = ctx.enter_context(tc.tile_pool(name="ps_mm", bufs=4, space="PSUM"))

        ident = consts.tile([P, P], f32, name="ident")
        make_identity(nc, ident)
        ones = consts.tile([1, P], bf16, name="ones")
        crow = consts.tile([1, 512], bf16, name="crow")
        cneg = consts.tile([P, 1], f32, name="cneg")
        nc.gpsimd.memset(ones, 1.0)
        nc.gpsimd.memset(crow, C)
        nc.gpsimd.memset(cneg, -C)

        # bias constant: rows rt*128+p at block rt (full tiles only).
        # One tile per rt (not one big tile): the Tile framework tracks
        # dependencies at tile granularity, so a single shared tile would make
        # the FIRST subtract wait for ALL bias DMAs. HWDGE/DMA_ENGINES are
        # serialized global devices, so issue order below matters too.
        cb_t = [consts.tile([P, N], f32, name=f"cb_{rt}") for rt in range(NRT)]
        cb6 = consts.tile([P, N], f32, name="cb6")

        def load_cb(rt):
            nc.sync.dma_start(
                out=cb_t[rt], in_=cb3.ap()[rt * P : (rt + 1) * P, :]
            )

        def load_cb6(b):
            # tail-tile bias: graph b occupies partitions 32b..32b+16
            nc.sync.dma_start(
                out=cb6[32 * b : 32 * b + 16, :], in_=cb3.ap()[NRT * P :, :]
            )

        staging = consts.tile([P, BPC * NRT * 16 + 16], f32, name="staging")
        praw6 = consts.tile([P, N], f32, name="praw6")
        praw_t = [dict() for _ in range(BPC)]
        xh_t = {}

        def topk(packed, out16):
            """per-partition top-10 of packed[*, 0:784] -> out16 (sorted 16)."""
            v16 = v_pool.tile([P, 16], f32, tag="v16")
            pv = packed.rearrange("p (m two) -> p two m", two=2)
            nc.vector.max(out=v16[:, 0:8], in_=pv[:, 0])
            nc.vector.max(out=v16[:, 8:16], in_=pv[:, 1])
            nc.vector.max(out=out16[:, 0:8], in_=v16)
            nc.vector.match_replace(
                out=v16, in_to_replace=out16[:, 0:8], in_values=v16,
                imm_value=-3.0e38,
            )
            nc.vector.max(out=out16[:, 8:16], in_=v16)

        def pe_warmup():
            # The cost model halves PE speed for ~3us after an idle gap.
            # Burn dummy rank-1 matmuls into a scratch psum bank while the
            # first xh DMA is in flight so the real matmuls start warm.
            wps = ps_mm.tile([P, 1024], f32, tag="ps_mm", name="warmup_ps")
            for i in range(N_WARMUP):
                nc.tensor.matmul(
                    wps[:, 0:512], lhsT=ones, rhs=crow,
                    start=True, stop=True,
                )

        def load_graph(b):
            # one tile per contraction k-block so the first matmuls only wait
            # on the first 1/4 of the transfer
            xk = []
            for k in range(4):
                t = xh_pool.tile([P, N], bf16, tag="xh", name=f"xh_{b}_{k}")
                nc.sync.dma_start(
                    out=t, in_=xhT.ap()[b, k * P : (k + 1) * P, :]
                )
                xk.append(t)
            xh_t[b] = xk

        def rt_tile(b, rt):
            xh = xh_t[b]
            t = b * NRT + rt
            n_mirror = rt // 2
            d0 = n_mirror * 256
            psum = ps_mm.tile([P, 1024], f32, tag="ps_mm")
            for c in range(n_mirror, 3):
                sl = psum[:, c * 256 : (c + 1) * 256]
                for k in range(4):
                    nc.tensor.matmul(
                        sl,
                        lhsT=xh[k][:, rt * P : rt * P + P],
                        rhs=xh[k][:, c * 256 : (c + 1) * 256],
                        start=(k == 0),
                        stop=False,
                    )
                nc.tensor.matmul(
                    sl, lhsT=ones, rhs=crow[:, 0:256], start=False, stop=True
                )
            sl = psum[:, 768:784]
            for k in range(4):
                nc.tensor.matmul(
                    sl,
                    lhsT=xh[k][:, rt * P : rt * P + P],
                    rhs=xh[k][:, 768:784],
                    start=(k == 0),
                    stop=False,
                )
            nc.tensor.matmul(sl, lhsT=ones, rhs=crow[:, 0:16], start=False, stop=True)

            praw = praw_pool.tile([P, N], f32, tag="praw", name=f"praw_{b}_{rt}")
            praw_t[b][rt] = praw
            # mirrored below-diagonal blocks transpose into the unused low
            # region of the same psum tile (no separate transpose-psum pool)
            for mi in range(2 * n_mirror):
                nc.tensor.transpose(
                    psum[:, mi * P : (mi + 1) * P],
                    praw_t[b][mi][:, rt * P : (rt + 1) * P],
                    ident,
                )
            if b == BPC - 1 and rt >= 4:
                # last graph's final tiles: emit the 16-wide tail slice first
                # so the rt6 mirror transpose (tail critical path) starts as
                # soon as the psum is ready
                nc.scalar.activation(
                    praw[:, 768:784], psum[:, 768:784], AF.Identity, bias=cneg
                )
                nc.scalar.activation(
                    praw[:, d0:768], psum[:, d0:768], AF.Identity, bias=cneg
                )
            else:
                nc.scalar.activation(
                    praw[:, d0:784], psum[:, d0:784], AF.Identity, bias=cneg
                )
            if n_mirror:
                nc.scalar.activation(praw[:, 0:d0], psum[:, 0:d0], AF.Copy)
            packed = packed_pool.tile([P, N], f32, tag="packed")
            nc.gpsimd.tensor_sub(
                packed[:, 0:SPLIT], praw[:, 0:SPLIT], cb_t[rt][:, 0:SPLIT]
            )
            nc.vector.tensor_sub(
                packed[:, SPLIT:784], praw[:, SPLIT:784], cb_t[rt][:, SPLIT:784]
            )
            topk(packed, staging[:, t * 16 : (t + 1) * 16])

        def rt6_graph(b):
            """tail rows 768:784 of graph b -> praw6[32b:32b+16]."""
            xh = xh_t[b]
            r0 = 32 * b
            ps6 = ps_mm.tile([P, 1024], f32, tag="ps_mm", name=f"ps6_{b}")
            for mt in range(NRT):
                nc.tensor.transpose(
                    ps6[:16, mt * P : (mt + 1) * P],
                    praw_t[b][mt][:, 768:784],
                    ident,
                )
            sl = ps6[:16, 768:784]
            for k in range(4):
                nc.tensor.matmul(
                    sl,
                    lhsT=xh[k][:, 768:784],
                    rhs=xh[k][:, 768:784],
                    start=(k == 0),
                    stop=False,
                )
            nc.tensor.matmul(
                sl, lhsT=ones[:, 0:16], rhs=crow[:, 0:16], start=False, stop=True
            )
            if b == BPC - 1:
                # split by mirror-source readiness: mt0-3 land early, mt5
                # after tile (b,5), mt4 (cols 512:640) is the tail critical
                # path -- keep its ACT slice minimal
                nc.scalar.activation(
                    praw6[r0 : r0 + 16, 0:512], ps6[:16, 0:512], AF.Copy
                )
                nc.scalar.activation(
                    praw6[r0 : r0 + 16, 640:768], ps6[:16, 640:768], AF.Copy
                )
                nc.scalar.activation(
                    praw6[r0 : r0 + 16, 512:640], ps6[:16, 512:640], AF.Copy
                )
            else:
                nc.scalar.activation(
                    praw6[r0 : r0 + 16, 0:768], ps6[:16, 0:768], AF.Copy
                )
            nc.scalar.activation(
                praw6[r0 : r0 + 16, 768:784], ps6[:16, 768:784],
                AF.Identity, bias=cneg[:16],
            )

        # DMA issue order matters: xh of graph 0 first so compute starts
        # immediately, bias blocks interleaved in consumption order.
        if N_WARMUP:
            pe_warmup()
        load_graph(0)
        load_cb(0)
        load_cb(1)
        load_graph(1)
        load_cb(2)
        load_cb(3)
        load_graph(2)
        load_cb(4)
        load_cb(5)
        load_graph(3)
        for b in range(BPC):
            load_cb6(b)
        for b in range(BPC):
            for rt in RT_ORDER:
                rt_tile(b, rt)
            rt6_graph(b)
            if b == 1:
                # first half of the results ships mid-run; the final DMA
                # then only moves 208 columns, shrinking the end epilogue
                nc.sync.dma_start(
                    out=res_out.ap()[:, 0:192], in_=staging[:, 0:192]
                )

        packed6 = packed_pool.tile([P, N], f32, tag="packed", name="packed6")
        nc.gpsimd.tensor_sub(
            packed6[:, 0:SPLIT6], praw6[:, 0:SPLIT6], cb6[:, 0:SPLIT6]
        )
        nc.vector.tensor_sub(
            packed6[:, SPLIT6:784], praw6[:, SPLIT6:784], cb6[:, SPLIT6:784]
        )
        topk(packed6, staging[:, BPC * NRT * 16 :])

        nc.sync.dma_start(out=res_out.ap()[:, 192:], in_=staging[:, 192:])

    nc.finalize()
    return nc


def _get_nc():
    if "nc" not in _CACHE:
        _CACHE["nc"] = build_bass()
    return _CACHE["nc"]


def kernel(node_feature, relative_pos):
    import ml_dtypes
    from concourse.bass_utils import run_bass_kernel_spmd

    x = np.asarray(node_feature, dtype=np.float32)
    rel = np.asarray(relative_pos, dtype=np.float32).reshape(N, N)

    # host prep: normalize, scale by sqrt(S), transpose to [D, N], bf16
    nrm = np.sqrt((x * x).sum(-1, dtype=np.float32), dtype=np.float32)
    nrm = np.maximum(nrm, np.float32(1e-12))
    xh = (x / nrm[..., None]) * np.float32(np.sqrt(S))
    xhT = np.ascontiguousarray(xh.transpose(0, 2, 1)).astype(ml_dtypes.bfloat16)

    # cb3 = q1024(S*cb) - (1023 - m),   cb = (rel + INF*mask)/2
    cb = (rel + np.float32(INF) * _mask_np()) * np.float32(0.5)
    q_cb = np.rint(cb.astype(np.float64) * S / 1024.0) * 1024.0
    r_m = (1023 - np.arange(N, dtype=np.float64))[None, :]
    cb3 = (q_cb - r_m).astype(np.float32)

    nc = _get_nc()
    in_maps = [
        {
            "xhT": np.ascontiguousarray(xhT[i * BPC : (i + 1) * BPC]),
            "cb3": cb3,
        }
        for i in range(NCORES)
    ]
    res = run_bass_kernel_spmd(nc, in_maps, list(range(NCORES)))

    # decode: packed -> column index, take top-10 of the sorted 16
    topk = np.zeros((BATCH, N, K), np.int32)
    for i in range(NCORES):
        r = res.results[i]["res"].astype(np.float64)  # [128, BPC*6*16 + 16]
        for b in range(BPC):
            g = i * BPC + b
            for rt in range(NRT):
                t = b * NRT + rt
                pk = r[:, t * 16 : t * 16 + K]
                topk[g, rt * P : (rt + 1) * P, :] = (
                    1023.0 - np.mod(pk, 1024.0)
                ).astype(np.int32)
            pk6 = r[32 * b : 32 * b + 16, BPC * NRT * 16 : BPC * NRT * 16 + K]
            topk[g, NRT * P :, :] = (1023.0 - np.mod(pk6, 1024.0)).astype(np.int32)

    dst = topk + (np.arange(BATCH, dtype=np.int32) * N)[:, None, None]
    src = np.broadcast_to(
        np.arange(BATCH * N, dtype=np.int32).reshape(BATCH, N, 1), (BATCH, N, K)
    )
    relation = np.zeros_like(dst)
    return np.stack([dst, src, relation], axis=-1).reshape(-1, 3)


# revision 67
# speedup vs baseline: 1.0233x; 1.0104x over previous
"""Trainium2 Bass kernel for nn_MediumRangeEdge (retrieval_knn).

For each batch graph: L2-normalize node features, pairwise score
score = sim - (rel + INF*mask)/2 (row/col constants dropped: ||x||=1),
top-10 largest per node, emit edge list [dst, src, 0].

Distribution: data-parallel over batch. 32 graphs -> 8 NeuronCores, 4
graphs per core. No cross-device communication.

Packed-score top-k (the key trick): the device packs the column index
into the low bits of the score so NO max_index / full-width
match_replace passes are needed:

    packed[n, m] = q1024(S*sim[n,m]) - q1024(S*cb[n,m]) + (1023 - m)

with S = 2^23 and q1024 = round-to-multiple-of-1024. All three terms are
exact integers < 2^24 in f32, so the low 10 bits carry the index and the
host decodes m = 1023 - (packed mod 1024). Ordering by packed ==
ordering by quantized score with ties broken low-index-first, matching
jax.lax.top_k. Quantization at delta = 1024/S = 1.2e-4 plus bf16 matmul
inputs scrambles only near-ties (measured rel_err ~1.8e-3, gate 2e-2).

How each term is produced:
  - PE: sim via bf16 matmuls (inputs pre-scaled by sqrt(S) on host, so
    PSUM = S*sim), then ONE extra rank-1 matmul row accumulates
    C = 1.5*2^33 LAST: the f32 add in the [2^33, 2^34) binade rounds
    S*sim to a multiple of 1024 (the quantizer is the PSUM itself).
  - ACT: Identity-activation copy PSUM->SBUF with bias = -C (per-
    partition AP) removes C exactly, leaving praw = q1024(S*sim).
  - Pool+DVE: the per-tile subtract of the host-built constant
    cb3[n, m] = q1024(S*cb[n,m]) - (1023 - m) is column-split at SPLIT:
    Pool takes cols [0:SPLIT] (Q7 software sub, 0.42 roofline), DVE the
    rest, sized so Pool's share equals DVE's share plus the top-k that
    only DVE can run -- all of PE/Pool/DVE sit within ~1.5us of each
    other (~35us busy each).
  - DVE top-k per 128-row tile: max8 over even columns, max8 over odd
    columns (spatial clusters split ~5/5 between parities so per-parity
    top-8 covers the global top-10 with ~0.1% tail misses), then
    max8 / match_replace / max8 on the 16 merged candidates. 5 DVE ops
    of which only two scan 392 elements -- ~1.2us/tile vs 4.4us for the
    classic 5-pass full-width top-k.

P = sim matrix is symmetric: only 256-wide column blocks not fully below
the diagonal are computed; fully-below blocks and the 16-row tail are
mirrored from earlier row tiles with PE transposes (praw is quantized
identically on both sides, so mirrors are bit-consistent).

Host does layout only: normalize + transpose + bf16-cast of the inputs,
the cb3 constant, and final index decode + edge-list assembly.
"""

import sys

if "/opt/trn_rl_repo" not in sys.path:
    sys.path.insert(0, "/opt/trn_rl_repo")

import numpy as np

BATCH = 32
N = 784  # 28*28 nodes
D = 512
K = 10
RES = 28
INF = 100000.0
NCORES = 8
BPC = BATCH // NCORES  # graphs per core

P = 128
NRT = 6  # full 128-row tiles; tail 16 rows handled packed across graphs
S = float(2.0**23)
C = float(1.5 * 2.0**33)  # binade [2^33, 2^34): f32 add rounds to 1024-multiples

# Every tile's packed-subtract is split at column SPLIT between Pool
# (cols [0:SPLIT], Q7 software sub at 0.42 roofline) and DVE (the rest).
# SPLIT is chosen so Pool's share matches DVE's share PLUS the 1.17us
# top-k that only DVE can run -- both engines stay saturated every tile.
SPLIT = 648
# tail-tile sub split (Pool part runs while DVE drains its topk backlog)
SPLIT6 = 648
# natural per-graph tile order (an rt5-before-rt4 variant helped an older
# structure but costs ~0.6us under the split-sub pipeline)
RT_ORDER = [0, 1, 2, 3, 4, 5]
# dummy 512-wide rank-1 matmuls at kernel start to warm the PE p-state model
N_WARMUP = 0

_CACHE = {}


def _mask_np():
    idx = np.arange(N)
    r, c = idx // RES, idx % RES
    mask = np.zeros((N, N), np.float32)
    for dr, dc in [(0, -1), (0, 1), (-1, 0), (1, 0), (-1, -1), (-1, 1), (1, -1), (1, 1)]:
        rr, cc = r + dr, c + dc
        valid = (rr >= 0) & (rr < RES) & (cc >= 0) & (cc < RES)
        mask[idx[valid], (rr * RES + cc)[valid]] = 1.0
    mask[idx, idx] = 1.0
    return mask


def build_bass():
    import concourse.bacc as bacc
    import concourse.mybir as mybir
    from concourse.tile import TileContext
    from concourse.masks import make_identity
    from contextlib import ExitStack

    f32 = mybir.dt.float32
    bf16 = mybir.dt.bfloat16
    AF = mybir.ActivationFunctionType
    AL = mybir.AluOpType

    nc = bacc.Bacc("TRN2", target_bir_lowering=False, debug=False, num_devices=NCORES)
    xhT = nc.declare_dram_parameter("xhT", [BPC, D, N], bf16, isOutput=False)
    cb3 = nc.declare_dram_parameter("cb3", [N, N], f32, isOutput=False)
    res_out = nc.declare_dram_parameter(
        "res", [P, BPC * NRT * 16 + 16], f32, isOutput=True
    )

    with TileContext(nc) as tc, ExitStack() as ctx:
        consts = ctx.enter_context(tc.tile_pool(name="consts", bufs=1))
        xh_pool = ctx.enter_context(tc.tile_pool(name="xh", bufs=16))
        praw_pool = ctx.enter_context(tc.tile_pool(name="praw", bufs=14))
        packed_pool = ctx.enter_context(tc.tile_pool(name="packed", bufs=6))
        v_pool = ctx.enter_context(tc.tile_pool(name="v16", bufs=10))
        ps_mm = ctx.enter_context(tc.tile_pool(name="ps_mm", bufs=4, space="PSUM"))

        ident = consts.tile([P, P], f32, name="ident")
        make_identity(nc, ident)
        ones = consts.tile([1, P], bf16, name="ones")
        crow = consts.tile([1, 512], bf16, name="crow")
        cneg = consts.tile([P, 1], f32, name="cneg")
        nc.gpsimd.memset(ones, 1.0)
        nc.gpsimd.memset(crow, C)
        nc.gpsimd.memset(cneg, -C)

        # bias constant: rows rt*128+p at block rt (full tiles only).
        # One tile per rt (not one big tile): the Tile framework tracks
        # dependencies at tile granularity, so a single shared tile would make
        # the FIRST subtract wait for ALL bias DMAs. HWDGE/DMA_ENGINES are
        # serialized global devices, so issue order below matters too.
        cb_t = [consts.tile([P, N], f32, name=f"cb_{rt}") for rt in range(NRT)]
        cb6 = consts.tile([P, N], f32, name="cb6")

        def load_cb(rt):
            nc.sync.dma_start(
                out=cb_t[rt], in_=cb3.ap()[rt * P : (rt + 1) * P, :]
            )

        def load_cb6(b):
            # tail-tile bias: graph b occupies partitions 32b..32b+16
            nc.sync.dma_start(
                out=cb6[32 * b : 32 * b + 16, :], in_=cb3.ap()[NRT * P :, :]
            )

        staging = consts.tile([P, BPC * NRT * 16 + 16], f32, name="staging")
        praw6 = consts.tile([P, N], f32, name="praw6")
        praw_t = [dict() for _ in range(BPC)]
        xh_t = {}

        def topk(packed, out16):
            """per-partition top-10 of packed[*, 0:784] -> out16 (sorted 16)."""
            v16 = v_pool.tile([P, 16], f32, tag="v16")
            pv = packed.rearrange("p (m two) -> p two m", two=2)
            nc.vector.max(out=v16[:, 0:8], in_=pv[:, 0])
            nc.vector.max(out=v16[:, 8:16], in_=pv[:, 1])
            nc.vector.max(out=out16[:, 0:8], in_=v16)
            nc.vector.match_replace(
                out=v16, in_to_replace=out16[:, 0:8], in_values=v16,
                imm_value=-3.0e38,
            )
            nc.vector.max(out=out16[:, 8:16], in_=v16)

        def pe_warmup():
            # The cost model halves PE speed for ~3us after an idle gap.
            # Burn dummy rank-1 matmuls into a scratch psum bank while the
            # first xh DMA is in flight so the real matmuls start warm.
            wps = ps_mm.tile([P, 1024], f32, tag="ps_mm", name="warmup_ps")
            for i in range(N_WARMUP):
                nc.tensor.matmul(
                    wps[:, 0:512], lhsT=ones, rhs=crow,
                    start=True, stop=True,
                )

        def load_graph(b):
            # one tile per contraction k-block so the first matmuls only wait
            # on the first 1/4 of the transfer
            xk = []
            for k in range(4):
                t = xh_pool.tile([P, N], bf16, tag="xh", name=f"xh_{b}_{k}")
                nc.sync.dma_start(
                    out=t, in_=xhT.ap()[b, k * P : (k + 1) * P, :]
                )
                xk.append(t)
            xh_t[b] = xk

        def rt_tile(b, rt):
            xh = xh_t[b]
            t = b * NRT + rt
            n_mirror = rt // 2
            d0 = n_mirror * 256
            psum = ps_mm.tile([P, 1024], f32, tag="ps_mm")
            for c in range(n_mirror, 3):
                sl = psum[:, c * 256 : (c + 1) * 256]
                for k in range(4):
                    nc.tensor.matmul(
                        sl,
                        lhsT=xh[k][:, rt * P : rt * P + P],
                        rhs=xh[k][:, c * 256 : (c + 1) * 256],
                        start=(k == 0),
                        stop=False,
                    )
                nc.tensor.matmul(
                    sl, lhsT=ones, rhs=crow[:, 0:256], start=False, stop=True
                )
            sl = psum[:, 768:784]
            for k in range(4):
                nc.tensor.matmul(
                    sl,
                    lhsT=xh[k][:, rt * P : rt * P + P],
                    rhs=xh[k][:, 768:784],
                    start=(k == 0),
                    stop=False,
                )
            nc.tensor.matmul(sl, lhsT=ones, rhs=crow[:, 0:16], start=False, stop=True)

            praw = praw_pool.tile([P, N], f32, tag="praw", name=f"praw_{b}_{rt}")
            praw_t[b][rt] = praw
            # mirrored below-diagonal blocks transpose into the unused low
            # region of the same psum tile (no separate transpose-psum pool)
            for mi in range(2 * n_mirror):
                nc.tensor.transpose(
                    psum[:, mi * P : (mi + 1) * P],
                    praw_t[b][mi][:, rt * P : (rt + 1) * P],
                    ident,
                )
            if False:
                # last graph's final tiles: emit the 16-wide tail slice first
                # so the rt6 mirror transpose (tail critical path) starts as
                # soon as the psum is ready
                nc.scalar.activation(
                    praw[:, 768:784], psum[:, 768:784], AF.Identity, bias=cneg
                )
                nc.scalar.activation(
                    praw[:, d0:768], psum[:, d0:768], AF.Identity, bias=cneg
                )
            else:
                nc.scalar.activation(
                    praw[:, d0:784], psum[:, d0:784], AF.Identity, bias=cneg
                )
            if n_mirror:
                nc.scalar.activation(praw[:, 0:d0], psum[:, 0:d0], AF.Copy)
            packed = packed_pool.tile([P, N], f32, tag="packed")
            nc.gpsimd.tensor_sub(
                packed[:, 0:SPLIT], praw[:, 0:SPLIT], cb_t[rt][:, 0:SPLIT]
            )
            nc.vector.tensor_sub(
                packed[:, SPLIT:784], praw[:, SPLIT:784], cb_t[rt][:, SPLIT:784]
            )
            topk(packed, staging[:, t * 16 : (t + 1) * 16])

        def rt6_graph(b):
            """tail rows 768:784 of graph b -> praw6[32b:32b+16]."""
            xh = xh_t[b]
            r0 = 32 * b
            ps6 = ps_mm.tile([P, 1024], f32, tag="ps_mm", name=f"ps6_{b}")
            for mt in range(NRT):
                nc.tensor.transpose(
                    ps6[:16, mt * P : (mt + 1) * P],
                    praw_t[b][mt][:, 768:784],
                    ident,
                )
            sl = ps6[:16, 768:784]
            for k in range(4):
                nc.tensor.matmul(
                    sl,
                    lhsT=xh[k][:, 768:784],
                    rhs=xh[k][:, 768:784],
                    start=(k == 0),
                    stop=False,
                )
            nc.tensor.matmul(
                sl, lhsT=ones[:, 0:16], rhs=crow[:, 0:16], start=False, stop=True
            )
            if b == BPC - 1:
                # split by mirror-source readiness: mt0-3 land early, mt5
                # after tile (b,5), mt4 (cols 512:640) is the tail critical
                # path -- keep its ACT slice minimal
                nc.scalar.activation(
                    praw6[r0 : r0 + 16, 0:512], ps6[:16, 0:512], AF.Copy
                )
                nc.scalar.activation(
                    praw6[r0 : r0 + 16, 640:768], ps6[:16, 640:768], AF.Copy
                )
                nc.scalar.activation(
                    praw6[r0 : r0 + 16, 512:640], ps6[:16, 512:640], AF.Copy
                )
            else:
                nc.scalar.activation(
                    praw6[r0 : r0 + 16, 0:768], ps6[:16, 0:768], AF.Copy
                )
            nc.scalar.activation(
                praw6[r0 : r0 + 16, 768:784], ps6[:16, 768:784],
                AF.Identity, bias=cneg[:16],
            )

        # DMA issue order matters: xh of graph 0 first so compute starts
        # immediately, bias blocks interleaved in consumption order.
        if N_WARMUP:
            pe_warmup()
        load_graph(0)
        load_cb(0)
        load_cb(1)
        load_graph(1)
        load_cb(2)
        load_cb(3)
        load_graph(2)
        load_cb(4)
        load_cb(5)
        load_graph(3)
        for b in range(BPC):
            load_cb6(b)
        for b in range(BPC):
            for rt in RT_ORDER:
                rt_tile(b, rt)
            rt6_graph(b)
            if b == 1:
                # first half of the results ships mid-run; the final DMA
                # then only moves 208 columns, shrinking the end epilogue
                nc.sync.dma_start(
                    out=res_out.ap()[:, 0:192], in_=staging[:, 0:192]
                )

        packed6 = packed_pool.tile([P, N], f32, tag="packed", name="packed6")
        nc.gpsimd.tensor_sub(
            packed6[:, 0:SPLIT6], praw6[:, 0:SPLIT6], cb6[:, 0:SPLIT6]
        )
        nc.vector.tensor_sub(
            packed6[:, SPLIT6:784], praw6[:, SPLIT6:784], cb6[:, SPLIT6:784]
        )
        topk(packed6, staging[:, BPC * NRT * 16 :])

        nc.sync.dma_start(out=res_out.ap()[:, 192:], in_=staging[:, 192:])

    nc.finalize()
    return nc


def _get_nc():
    if "nc" not in _CACHE:
        _CACHE["nc"] = build_bass()
    return _CACHE["nc"]


def kernel(node_feature, relative_pos):
    import ml_dtypes
    from concourse.bass_utils import run_bass_kernel_spmd

    x = np.asarray(node_feature, dtype=np.float32)
    rel = np.asarray(relative_pos, dtype=np.float32).reshape(N, N)

    # host prep: normalize, scale by sqrt(S), transpose to [D, N], bf16
    nrm = np.sqrt((x * x).sum(-1, dtype=np.float32), dtype=np.float32)
    nrm = np.maximum(nrm, np.float32(1e-12))
    xh = (x / nrm[..., None]) * np.float32(np.sqrt(S))
    xhT = np.ascontiguousarray(xh.transpose(0, 2, 1)).astype(ml_dtypes.bfloat16)

    # cb3 = q1024(S*cb) - (1023 - m),   cb = (rel + INF*mask)/2
    cb = (rel + np.float32(INF) * _mask_np()) * np.float32(0.5)
    q_cb = np.rint(cb.astype(np.float64) * S / 1024.0) * 1024.0
    r_m = (1023 - np.arange(N, dtype=np.float64))[None, :]
    cb3 = (q_cb - r_m).astype(np.float32)

    nc = _get_nc()
    in_maps = [
        {
            "xhT": np.ascontiguousarray(xhT[i * BPC : (i + 1) * BPC]),
            "cb3": cb3,
        }
        for i in range(NCORES)
    ]
    res = run_bass_kernel_spmd(nc, in_maps, list(range(NCORES)))

    # decode: packed -> column index, take top-10 of the sorted 16
    topk = np.zeros((BATCH, N, K), np.int32)
    for i in range(NCORES):
        r = res.results[i]["res"].astype(np.float64)  # [128, BPC*6*16 + 16]
        for b in range(BPC):
            g = i * BPC + b
            for rt in range(NRT):
                t = b * NRT + rt
                pk = r[:, t * 16 : t * 16 + K]
                topk[g, rt * P : (rt + 1) * P, :] = (
                    1023.0 - np.mod(pk, 1024.0)
                ).astype(np.int32)
            pk6 = r[32 * b : 32 * b + 16, BPC * NRT * 16 : BPC * NRT * 16 + K]
            topk[g, NRT * P :, :] = (1023.0 - np.mod(pk6, 1024.0)).astype(np.int32)

    dst = topk + (np.arange(BATCH, dtype=np.int32) * N)[:, None, None]
    src = np.broadcast_to(
        np.arange(BATCH * N, dtype=np.int32).reshape(BATCH, N, 1), (BATCH, N, K)
    )
    relation = np.zeros_like(dst)
    return np.stack([dst, src, relation], axis=-1).reshape(-1, 3)


# revision 70
# speedup vs baseline: 1.0241x; 1.0007x over previous
"""Trainium2 Bass kernel for nn_MediumRangeEdge (retrieval_knn).

For each batch graph: L2-normalize node features, pairwise score
score = sim - (rel + INF*mask)/2 (row/col constants dropped: ||x||=1),
top-10 largest per node, emit edge list [dst, src, 0].

Distribution: data-parallel over batch. 32 graphs -> 8 NeuronCores, 4
graphs per core. No cross-device communication.

Packed-score top-k (the key trick): the device packs the column index
into the low bits of the score so NO max_index / full-width
match_replace passes are needed:

    packed[n, m] = q1024(S*sim[n,m]) - q1024(S*cb[n,m]) + (1023 - m)

with S = 2^23 and q1024 = round-to-multiple-of-1024. All three terms are
exact integers < 2^24 in f32, so the low 10 bits carry the index and the
host decodes m = 1023 - (packed mod 1024). Ordering by packed ==
ordering by quantized score with ties broken low-index-first, matching
jax.lax.top_k. Quantization at delta = 1024/S = 1.2e-4 plus bf16 matmul
inputs scrambles only near-ties (measured rel_err ~1.8e-3, gate 2e-2).

How each term is produced:
  - PE: sim via bf16 matmuls (inputs pre-scaled by sqrt(S) on host, so
    PSUM = S*sim), then ONE extra rank-1 matmul row accumulates
    C = 1.5*2^33 LAST: the f32 add in the [2^33, 2^34) binade rounds
    S*sim to a multiple of 1024 (the quantizer is the PSUM itself).
  - ACT: Identity-activation copy PSUM->SBUF with bias = -C (per-
    partition AP) removes C exactly, leaving praw = q1024(S*sim).
  - Pool+DVE: the per-tile subtract of the host-built constant
    cb3[n, m] = q1024(S*cb[n,m]) - (1023 - m) is column-split at SPLIT:
    Pool takes cols [0:SPLIT] (Q7 software sub, 0.42 roofline), DVE the
    rest, sized so Pool's share equals DVE's share plus the top-k that
    only DVE can run -- all of PE/Pool/DVE sit within ~1.5us of each
    other (~35us busy each).
  - DVE top-k per 128-row tile: max8 over even columns, max8 over odd
    columns (spatial clusters split ~5/5 between parities so per-parity
    top-8 covers the global top-10 with ~0.1% tail misses), then
    max8 / match_replace / max8 on the 16 merged candidates. 5 DVE ops
    of which only two scan 392 elements -- ~1.2us/tile vs 4.4us for the
    classic 5-pass full-width top-k.

P = sim matrix is symmetric: only 256-wide column blocks not fully below
the diagonal are computed; fully-below blocks and the 16-row tail are
mirrored from earlier row tiles with PE transposes (praw is quantized
identically on both sides, so mirrors are bit-consistent).

Host does layout only: normalize + transpose + bf16-cast of the inputs,
the cb3 constant, and final index decode + edge-list assembly.
"""

import sys

if "/opt/trn_rl_repo" not in sys.path:
    sys.path.insert(0, "/opt/trn_rl_repo")

import numpy as np

BATCH = 32
N = 784  # 28*28 nodes
D = 512
K = 10
RES = 28
INF = 100000.0
NCORES = 8
BPC = BATCH // NCORES  # graphs per core

P = 128
NRT = 6  # full 128-row tiles; tail 16 rows handled packed across graphs
S = float(2.0**23)
C = float(1.5 * 2.0**33)  # binade [2^33, 2^34): f32 add rounds to 1024-multiples

# Every tile's packed-subtract is split at column SPLIT between Pool
# (cols [0:SPLIT], Q7 software sub at 0.42 roofline) and DVE (the rest).
# SPLIT is chosen so Pool's share matches DVE's share PLUS the 1.17us
# top-k that only DVE can run -- both engines stay saturated every tile.
SPLIT = 648
# tail-tile sub split (Pool part runs while DVE drains its topk backlog)
SPLIT6 = 648
# near-natural per-graph tile order; rt1-first shaves a final ~35ns in the
# scheduler (rt1 has no mirror deps either, so any of the first two may lead)
RT_ORDER = [1, 0, 2, 3, 4, 5]
# dummy 512-wide rank-1 matmuls at kernel start to warm the PE p-state model
N_WARMUP = 0

_CACHE = {}


def _mask_np():
    idx = np.arange(N)
    r, c = idx // RES, idx % RES
    mask = np.zeros((N, N), np.float32)
    for dr, dc in [(0, -1), (0, 1), (-1, 0), (1, 0), (-1, -1), (-1, 1), (1, -1), (1, 1)]:
        rr, cc = r + dr, c + dc
        valid = (rr >= 0) & (rr < RES) & (cc >= 0) & (cc < RES)
        mask[idx[valid], (rr * RES + cc)[valid]] = 1.0
    mask[idx, idx] = 1.0
    return mask


def build_bass():
    import concourse.bacc as bacc
    import concourse.mybir as mybir
    from concourse.tile import TileContext
    from concourse.masks import make_identity
    from contextlib import ExitStack

    f32 = mybir.dt.float32
    bf16 = mybir.dt.bfloat16
    AF = mybir.ActivationFunctionType
    AL = mybir.AluOpType

    nc = bacc.Bacc("TRN2", target_bir_lowering=False, debug=False, num_devices=NCORES)
    xhT = nc.declare_dram_parameter("xhT", [BPC, D, N], bf16, isOutput=False)
    cb3 = nc.declare_dram_parameter("cb3", [N, N], f32, isOutput=False)
    res_out = nc.declare_dram_parameter(
        "res", [P, BPC * NRT * 16 + 16], f32, isOutput=True
    )

    with TileContext(nc) as tc, ExitStack() as ctx:
        consts = ctx.enter_context(tc.tile_pool(name="consts", bufs=1))
        xh_pool = ctx.enter_context(tc.tile_pool(name="xh", bufs=16))
        praw_pool = ctx.enter_context(tc.tile_pool(name="praw", bufs=14))
        packed_pool = ctx.enter_context(tc.tile_pool(name="packed", bufs=6))
        v_pool = ctx.enter_context(tc.tile_pool(name="v16", bufs=10))
        ps_mm = ctx.enter_context(tc.tile_pool(name="ps_mm", bufs=4, space="PSUM"))

        ident = consts.tile([P, P], f32, name="ident")
        make_identity(nc, ident)
        ones = consts.tile([1, P], bf16, name="ones")
        crow = consts.tile([1, 512], bf16, name="crow")
        cneg = consts.tile([P, 1], f32, name="cneg")
        nc.gpsimd.memset(ones, 1.0)
        nc.gpsimd.memset(crow, C)
        nc.gpsimd.memset(cneg, -C)

        # bias constant: rows rt*128+p at block rt (full tiles only).
        # One tile per rt (not one big tile): the Tile framework tracks
        # dependencies at tile granularity, so a single shared tile would make
        # the FIRST subtract wait for ALL bias DMAs. HWDGE/DMA_ENGINES are
        # serialized global devices, so issue order below matters too.
        cb_t = [consts.tile([P, N], f32, name=f"cb_{rt}") for rt in range(NRT)]
        cb6 = consts.tile([P, N], f32, name="cb6")

        def load_cb(rt):
            nc.sync.dma_start(
                out=cb_t[rt], in_=cb3.ap()[rt * P : (rt + 1) * P, :]
            )

        def load_cb6(b):
            # tail-tile bias: graph b occupies partitions 32b..32b+16
            nc.sync.dma_start(
                out=cb6[32 * b : 32 * b + 16, :], in_=cb3.ap()[NRT * P :, :]
            )

        staging = consts.tile([P, BPC * NRT * 16 + 16], f32, name="staging")
        praw6 = consts.tile([P, N], f32, name="praw6")
        praw_t = [dict() for _ in range(BPC)]
        xh_t = {}

        def topk(packed, out16):
            """per-partition top-10 of packed[*, 0:784] -> out16 (sorted 16)."""
            v16 = v_pool.tile([P, 16], f32, tag="v16")
            pv = packed.rearrange("p (m two) -> p two m", two=2)
            nc.vector.max(out=v16[:, 0:8], in_=pv[:, 0])
            nc.vector.max(out=v16[:, 8:16], in_=pv[:, 1])
            nc.vector.max(out=out16[:, 0:8], in_=v16)
            nc.vector.match_replace(
                out=v16, in_to_replace=out16[:, 0:8], in_values=v16,
                imm_value=-3.0e38,
            )
            nc.vector.max(out=out16[:, 8:16], in_=v16)

        def pe_warmup():
            # The cost model halves PE speed for ~3us after an idle gap.
            # Burn dummy rank-1 matmuls into a scratch psum bank while the
            # first xh DMA is in flight so the real matmuls start warm.
            wps = ps_mm.tile([P, 1024], f32, tag="ps_mm", name="warmup_ps")
            for i in range(N_WARMUP):
                nc.tensor.matmul(
                    wps[:, 0:512], lhsT=ones, rhs=crow,
                    start=True, stop=True,
                )

        def load_graph(b):
            # one tile per contraction k-block so the first matmuls only wait
            # on the first 1/4 of the transfer
            xk = []
            for k in range(4):
                t = xh_pool.tile([P, N], bf16, tag="xh", name=f"xh_{b}_{k}")
                nc.sync.dma_start(
                    out=t, in_=xhT.ap()[b, k * P : (k + 1) * P, :]
                )
                xk.append(t)
            xh_t[b] = xk

        def rt_tile(b, rt):
            xh = xh_t[b]
            t = b * NRT + rt
            n_mirror = rt // 2
            d0 = n_mirror * 256
            psum = ps_mm.tile([P, 1024], f32, tag="ps_mm")
            for c in range(n_mirror, 3):
                sl = psum[:, c * 256 : (c + 1) * 256]
                for k in range(4):
                    nc.tensor.matmul(
                        sl,
                        lhsT=xh[k][:, rt * P : rt * P + P],
                        rhs=xh[k][:, c * 256 : (c + 1) * 256],
                        start=(k == 0),
                        stop=False,
                    )
                nc.tensor.matmul(
                    sl, lhsT=ones, rhs=crow[:, 0:256], start=False, stop=True
                )
            sl = psum[:, 768:784]
            for k in range(4):
                nc.tensor.matmul(
                    sl,
                    lhsT=xh[k][:, rt * P : rt * P + P],
                    rhs=xh[k][:, 768:784],
                    start=(k == 0),
                    stop=False,
                )
            nc.tensor.matmul(sl, lhsT=ones, rhs=crow[:, 0:16], start=False, stop=True)

            praw = praw_pool.tile([P, N], f32, tag="praw", name=f"praw_{b}_{rt}")
            praw_t[b][rt] = praw
            # mirrored below-diagonal blocks transpose into the unused low
            # region of the same psum tile (no separate transpose-psum pool)
            for mi in range(2 * n_mirror):
                nc.tensor.transpose(
                    psum[:, mi * P : (mi + 1) * P],
                    praw_t[b][mi][:, rt * P : (rt + 1) * P],
                    ident,
                )
            if False:
                # last graph's final tiles: emit the 16-wide tail slice first
                # so the rt6 mirror transpose (tail critical path) starts as
                # soon as the psum is ready
                nc.scalar.activation(
                    praw[:, 768:784], psum[:, 768:784], AF.Identity, bias=cneg
                )
                nc.scalar.activation(
                    praw[:, d0:768], psum[:, d0:768], AF.Identity, bias=cneg
                )
            else:
                nc.scalar.activation(
                    praw[:, d0:784], psum[:, d0:784], AF.Identity, bias=cneg
                )
            if n_mirror:
                nc.scalar.activation(praw[:, 0:d0], psum[:, 0:d0], AF.Copy)
            packed = packed_pool.tile([P, N], f32, tag="packed")
            nc.gpsimd.tensor_sub(
                packed[:, 0:SPLIT], praw[:, 0:SPLIT], cb_t[rt][:, 0:SPLIT]
            )
            nc.vector.tensor_sub(
                packed[:, SPLIT:784], praw[:, SPLIT:784], cb_t[rt][:, SPLIT:784]
            )
            topk(packed, staging[:, t * 16 : (t + 1) * 16])

        def rt6_graph(b):
            """tail rows 768:784 of graph b -> praw6[32b:32b+16]."""
            xh = xh_t[b]
            r0 = 32 * b
            ps6 = ps_mm.tile([P, 1024], f32, tag="ps_mm", name=f"ps6_{b}")
            for mt in range(NRT):
                nc.tensor.transpose(
                    ps6[:16, mt * P : (mt + 1) * P],
                    praw_t[b][mt][:, 768:784],
                    ident,
                )
            sl = ps6[:16, 768:784]
            for k in range(4):
                nc.tensor.matmul(
                    sl,
                    lhsT=xh[k][:, 768:784],
                    rhs=xh[k][:, 768:784],
                    start=(k == 0),
                    stop=False,
                )
            nc.tensor.matmul(
                sl, lhsT=ones[:, 0:16], rhs=crow[:, 0:16], start=False, stop=True
            )
            if b == BPC - 1:
                # split by mirror-source readiness: mt0-3 land early, mt5
                # after tile (b,5), mt4 (cols 512:640) is the tail critical
                # path -- keep its ACT slice minimal
                nc.scalar.activation(
                    praw6[r0 : r0 + 16, 0:512], ps6[:16, 0:512], AF.Copy
                )
                nc.scalar.activation(
                    praw6[r0 : r0 + 16, 640:768], ps6[:16, 640:768], AF.Copy
                )
                nc.scalar.activation(
                    praw6[r0 : r0 + 16, 512:640], ps6[:16, 512:640], AF.Copy
                )
            else:
                nc.scalar.activation(
                    praw6[r0 : r0 + 16, 0:768], ps6[:16, 0:768], AF.Copy
                )
            nc.scalar.activation(
                praw6[r0 : r0 + 16, 768:784], ps6[:16, 768:784],
                AF.Identity, bias=cneg[:16],
            )

        # DMA issue order matters: xh of graph 0 first so compute starts
        # immediately, bias blocks interleaved in consumption order.
        if N_WARMUP:
            pe_warmup()
        load_graph(0)
        load_cb(0)
        load_cb(1)
        load_graph(1)
        load_cb(2)
        load_cb(3)
        load_graph(2)
        load_cb(4)
        load_cb(5)
        load_graph(3)
        for b in range(BPC):
            load_cb6(b)
        for b in range(BPC):
            for rt in RT_ORDER:
                rt_tile(b, rt)
            rt6_graph(b)
            if b == 1:
                # first half of the results ships mid-run; the final DMA
                # then only moves 208 columns, shrinking the end epilogue
                nc.sync.dma_start(
                    out=res_out.ap()[:, 0:192], in_=staging[:, 0:192]
                )

        packed6 = packed_pool.tile([P, N], f32, tag="packed", name="packed6")
        nc.gpsimd.tensor_sub(
            packed6[:, 0:SPLIT6], praw6[:, 0:SPLIT6], cb6[:, 0:SPLIT6]
        )
        nc.vector.tensor_sub(
            packed6[:, SPLIT6:784], praw6[:, SPLIT6:784], cb6[:, SPLIT6:784]
        )
        topk(packed6, staging[:, BPC * NRT * 16 :])

        nc.sync.dma_start(out=res_out.ap()[:, 192:], in_=staging[:, 192:])

    nc.finalize()
    return nc


def _get_nc():
    if "nc" not in _CACHE:
        _CACHE["nc"] = build_bass()
    return _CACHE["nc"]


def kernel(node_feature, relative_pos):
    import ml_dtypes
    from concourse.bass_utils import run_bass_kernel_spmd

    x = np.asarray(node_feature, dtype=np.float32)
    rel = np.asarray(relative_pos, dtype=np.float32).reshape(N, N)

    # host prep: normalize, scale by sqrt(S), transpose to [D, N], bf16
    nrm = np.sqrt((x * x).sum(-1, dtype=np.float32), dtype=np.float32)
    nrm = np.maximum(nrm, np.float32(1e-12))
    xh = (x / nrm[..., None]) * np.float32(np.sqrt(S))
    xhT = np.ascontiguousarray(xh.transpose(0, 2, 1)).astype(ml_dtypes.bfloat16)

    # cb3 = q1024(S*cb) - (1023 - m),   cb = (rel + INF*mask)/2
    cb = (rel + np.float32(INF) * _mask_np()) * np.float32(0.5)
    q_cb = np.rint(cb.astype(np.float64) * S / 1024.0) * 1024.0
    r_m = (1023 - np.arange(N, dtype=np.float64))[None, :]
    cb3 = (q_cb - r_m).astype(np.float32)

    nc = _get_nc()
    in_maps = [
        {
            "xhT": np.ascontiguousarray(xhT[i * BPC : (i + 1) * BPC]),
            "cb3": cb3,
        }
        for i in range(NCORES)
    ]
    res = run_bass_kernel_spmd(nc, in_maps, list(range(NCORES)))

    # decode: packed -> column index, take top-10 of the sorted 16
    topk = np.zeros((BATCH, N, K), np.int32)
    for i in range(NCORES):
        r = res.results[i]["res"].astype(np.float64)  # [128, BPC*6*16 + 16]
        for b in range(BPC):
            g = i * BPC + b
            for rt in range(NRT):
                t = b * NRT + rt
                pk = r[:, t * 16 : t * 16 + K]
                topk[g, rt * P : (rt + 1) * P, :] = (
                    1023.0 - np.mod(pk, 1024.0)
                ).astype(np.int32)
            pk6 = r[32 * b : 32 * b + 16, BPC * NRT * 16 : BPC * NRT * 16 + K]
            topk[g, NRT * P :, :] = (1023.0 - np.mod(pk6, 1024.0)).astype(np.int32)

    dst = topk + (np.arange(BATCH, dtype=np.int32) * N)[:, None, None]
    src = np.broadcast_to(
        np.arange(BATCH * N, dtype=np.int32).reshape(BATCH, N, 1), (BATCH, N, K)
    )
    relation = np.zeros_like(dst)
    return np.stack([dst, src, relation], axis=-1).reshape(-1, 3)
